# revision 1
# baseline (speedup 1.0000x reference)
"""Trainium2 Bass kernel for Llama GQA attention (no mask), 8-way tensor
parallel over KV heads.

Problem shapes (hardcoded):
  x  (2, 2048, 4096) f32
  wq (4096, 4096), wk (1024, 4096), wv (1024, 4096), wo (4096, 4096) f32
  NUM_HEADS=32, NUM_KV_HEADS=8, HEAD_DIM=128, GQA group g=4

Sharding: core c owns KV head c (4 Q heads). x replicated (pre-transposed
to xT on host), wq/wk/wv sharded on output dim (pre-transposed host-side),
wo sharded on input dim. Each core computes a partial (4096, 4096) output
(its heads' contribution through wo); host sums the 8 partials.

All matmuls run in fp32r (full-rate fp32, HIGH mode single pass).

Structure:
  phase 1: q/k/v projections. Weights DMA'd in per-k-tile chunks on the
    gpsimd queue (x tiles on the sync queue) so the first matmul starts
    ~3us in. vT -> v via PE transposes. PSUM j-boundary copies split
    across ACT and DVE to shorten the bank-reuse stall.
  phase 2 (fused attention + output projection, software-pipelined):
    per (batch, tq-chunk): for each of 4 heads: scores transposed
    ST = kT_tile.T @ qT chunk into [128,1024] PSUM (2 k-tiles), one
    batched exp (no max subtraction -- scores are bounded), PV + ones
    matmul denominator accumulated in PSUM, reciprocal_approx_fast +
    mul -> attnT chunk; then the *previous* chunk's output projection
    (accumulate 4 heads in PSUM against resident woT) so DVE latency
    and the wo DMA hide under compute.
"""

import sys
from contextlib import ExitStack

import numpy as np

sys.path.insert(0, "/opt/trn_rl_repo")

import concourse.bass as bass  # noqa: E402
import concourse.tile as tile  # noqa: E402
from concourse import bacc, mybir  # noqa: E402
from concourse.bass_utils import run_bass_kernel_spmd  # noqa: E402
from concourse.masks import make_identity  # noqa: E402

NCORES = 8
B, S, H = 2, 2048, 4096
T = B * S                      # 4096 flattened tokens
D = 128                        # head dim
G = 4                          # q heads per core (GQA group)
HK = 32                        # h k-tiles (4096 / 128)
TT = T // 128                  # 32 token tiles
NJ = T // 512                  # 8 token chunks of 512
SJ = S // 512                  # 4 tq chunks per batch
SI = S // 128                  # 16 tk tiles per batch
SCALE = float(1.0 / np.sqrt(D))

F32 = mybir.dt.float32
F32R = mybir.dt.float32r
COPY = mybir.ActivationFunctionType.Copy
EXP = mybir.ActivationFunctionType.Exp


def build_nc():
    nc = bacc.Bacc("TRN2", target_bir_lowering=False, debug=False,
                   enable_asserts=True, num_devices=NCORES)
    xt = nc.declare_dram_parameter("xt", [H, T], F32R, isOutput=False)
    wqt = nc.declare_dram_parameter("wqt", [H, G * D], F32R, isOutput=False)
    wkt = nc.declare_dram_parameter("wkt", [H, D], F32R, isOutput=False)
    wvt = nc.declare_dram_parameter("wvt", [H, D], F32R, isOutput=False)
    wot = nc.declare_dram_parameter("wot", [G * D, H], F32R, isOutput=False)
    ones = nc.declare_dram_parameter("ones", [128, 128], F32R, isOutput=False)
    out = nc.declare_dram_parameter("out", [T, H], F32, isOutput=True)

    xt_r = xt.ap().rearrange("(k p) t -> p k t", p=128)     # [128, 32, T]
    wqt_r = wqt.ap().rearrange("(k p) m -> p k m", p=128)   # [128, 32, 512]
    wkt_r = wkt.ap().rearrange("(k p) m -> p k m", p=128)   # [128, 32, 128]
    wvt_r = wvt.ap().rearrange("(k p) m -> p k m", p=128)   # [128, 32, 128]
    wot_r = wot.ap().rearrange("(k p) n -> p k n", p=128)   # [128, 4, T]
    out_r = out.ap()

    with tile.TileContext(nc) as tc:
        with ExitStack() as ctx:
            persist = ctx.enter_context(tc.tile_pool(name="persist", bufs=1))
            q_sb = persist.tile([128, G, T], F32R)       # qT per head, 8MB
            k_sb = persist.tile([128, T], F32R)          # kT, 2MB
            v_sb = persist.tile([128, TT, D], F32R)      # v natural, 2MB
            ones_sb = persist.tile([128, 128], F32R)
            nc.sync.dma_start(out=ones_sb, in_=ones.ap())

            # ---------------- phase 1: projections ----------------
            with ExitStack() as c1:
                wpool = c1.enter_context(tc.tile_pool(name="wpool", bufs=1))
                xpool = c1.enter_context(tc.tile_pool(name="xpool", bufs=4))
                vstg = c1.enter_context(tc.tile_pool(name="vstg", bufs=2))
                ps1 = c1.enter_context(tc.tile_pool(name="ps1", bufs=1, space="PSUM"))
                pstr = c1.enter_context(tc.tile_pool(name="pstr", bufs=2, space="PSUM"))

                wq_t = wpool.tile([128, HK, G * D], F32R)   # 8MB
                wk_t = wpool.tile([128, HK, D], F32R)       # 2MB
                wv_t = wpool.tile([128, HK, D], F32R)       # 2MB
                ident = wpool.tile([128, 128], F32)
                # chunk weight loads per k-tile on the gpsimd queue so the
                # first matmul's stationary arrives within ~1us
                for k in range(HK):
                    nc.gpsimd.dma_start(out=wq_t[:, k, :], in_=wqt_r[:, k, :])
                    nc.gpsimd.dma_start(out=wk_t[:, k, :], in_=wkt_r[:, k, :])
                    nc.gpsimd.dma_start(out=wv_t[:, k, :], in_=wvt_r[:, k, :])
                make_identity(nc, ident)

                def v_transpose(pj, pv_st):
                    # one-j-delayed so PE never waits on the DVE staging copy
                    vt_ps = pstr.tile([128, 4, 128], F32)
                    for tt in range(4):
                        nc.tensor.transpose(
                            vt_ps[:, tt, :], pv_st[:, tt * 128:(tt + 1) * 128],
                            ident)
                    nc.scalar.activation(
                        out=v_sb[:, 4 * pj:4 * pj + 4, :], in_=vt_ps, func=COPY)

                prev_v = None
                for j in range(NJ):
                    tsl = slice(j * 512, (j + 1) * 512)
                    q_ps = [ps1.tile([128, 512], F32, name=f"q_ps{m}")
                            for m in range(G)]
                    k_ps = ps1.tile([128, 512], F32)
                    v_ps = ps1.tile([128, 512], F32)
                    for k in range(HK):
                        x_t = xpool.tile([128, 512], F32R)
                        nc.sync.dma_start(out=x_t, in_=xt_r[:, k, tsl])
                        st = k == 0
                        sp = k == HK - 1
                        for m in range(G):
                            nc.tensor.matmul(
                                q_ps[m], wq_t[:, k, m * D:(m + 1) * D], x_t,
                                start=st, stop=sp)
                        nc.tensor.matmul(k_ps, wk_t[:, k, :], x_t, start=st, stop=sp)
                        nc.tensor.matmul(v_ps, wv_t[:, k, :], x_t, start=st, stop=sp)
                        if k == 2 and prev_v is not None:
                            v_transpose(*prev_v)
                    # split psum evacuation across ACT and DVE so the banks
                    # free up fast for the next j iteration
                    nc.scalar.activation(out=q_sb[:, 0, tsl], in_=q_ps[0], func=COPY)
                    nc.vector.tensor_copy(q_sb[:, 1, tsl], q_ps[1])
                    nc.scalar.activation(out=q_sb[:, 2, tsl], in_=q_ps[2], func=COPY)
                    nc.vector.tensor_copy(q_sb[:, 3, tsl], q_ps[3])
                    nc.scalar.activation(out=k_sb[:, tsl], in_=k_ps, func=COPY)
                    # v: vT [dv, t] -> transpose 128-col blocks -> v [t, dv]
                    v_st = vstg.tile([128, 512], F32)
                    nc.vector.tensor_copy(v_st, v_ps)
                    prev_v = (j, v_st)
                v_transpose(*prev_v)

            # ------- phase 2: fused attention + output projection -------
            with ExitStack() as c2:
                wopool = c2.enter_context(tc.tile_pool(name="wopool", bufs=1))
                apool = c2.enter_context(tc.tile_pool(name="apool", bufs=2))
                ppool = c2.enter_context(tc.tile_pool(name="ppool", bufs=3))
                rpool = c2.enter_context(tc.tile_pool(name="rpool", bufs=2))
                opool = c2.enter_context(tc.tile_pool(name="opool", bufs=3))
                psS = c2.enter_context(tc.tile_pool(name="psS", bufs=2, space="PSUM"))
                psPV = c2.enter_context(tc.tile_pool(name="psPV", bufs=1, space="PSUM"))
                psO = c2.enter_context(tc.tile_pool(name="psO", bufs=2, space="PSUM"))

                wo_sb = wopool.tile([128, G, T], F32R)      # 8MB resident
                for k in range(G):
                    nc.gpsimd.dma_start(out=wo_sb[:, k, :], in_=wot_r[:, k, :])

                def outproj(bj):
                    pb, pj, pa = bj
                    for tt2 in range(4):
                        t0 = pb * S + pj * 512 + tt2 * 128
                        for n in range(NJ):
                            o_ps = psO.tile([128, 512], F32)
                            for m in range(G):
                                nc.tensor.matmul(
                                    o_ps, pa[m][:, tt2 * 128:(tt2 + 1) * 128],
                                    wo_sb[:, m, n * 512:(n + 1) * 512],
                                    start=(m == 0), stop=(m == G - 1))
                            o_t = opool.tile([128, 512], F32)
                            nc.scalar.activation(out=o_t, in_=o_ps, func=COPY)
                            nc.sync.dma_start(
                                out=out_r[t0:t0 + 128, n * 512:(n + 1) * 512],
                                in_=o_t)

                prev = None
                for b in range(B):
                    for j in range(SJ):
                        tqsl = slice(b * S + j * 512, b * S + (j + 1) * 512)
                        a_ch = [apool.tile([128, 512], F32R, name=f"a_ch{m}")
                                for m in range(G)]
                        for m in range(G):
                            pv_ps = psPV.tile([128, 512], F32)
                            den_ps = psPV.tile([128, 512], F32)
                            for g in range(SI // 2):
                                s_ps = psS.tile([128, 1024], F32)
                                for h in range(2):
                                    ti = b * SI + 2 * g + h
                                    nc.tensor.matmul(
                                        s_ps[:, h * 512:(h + 1) * 512],
                                        k_sb[:, ti * 128:(ti + 1) * 128],
                                        q_sb[:, m, tqsl], start=True, stop=True)
                                p_t = ppool.tile([128, 1024], F32R)
                                nc.scalar.activation(out=p_t, in_=s_ps, func=EXP,
                                                     scale=SCALE)
                                for h in range(2):
                                    ti = b * SI + 2 * g + h
                                    st = g == 0 and h == 0
                                    sp = g == SI // 2 - 1 and h == 1
                                    nc.tensor.matmul(
                                        pv_ps, v_sb[:, ti, :],
                                        p_t[:, h * 512:(h + 1) * 512],
                                        start=st, stop=sp)
                                    nc.tensor.matmul(
                                        den_ps, ones_sb,
                                        p_t[:, h * 512:(h + 1) * 512],
                                        start=st, stop=sp)
                            rec_t = rpool.tile([128, 512], F32)
                            nc.vector.reciprocal_approx_fast(out=rec_t, in_=den_ps)
                            nc.vector.tensor_mul(a_ch[m], pv_ps, rec_t)
                        if prev is not None:
                            outproj(prev)
                        prev = (b, j, a_ch)
                outproj(prev)
    nc.compile()
    return nc


_NC_CACHE = None


def _get_nc():
    global _NC_CACHE
    if _NC_CACHE is None:
        _NC_CACHE = build_nc()
    return _NC_CACHE


def make_in_maps(x, wq, wk, wv, wo):
    xt = np.ascontiguousarray(x.reshape(T, H).T)
    ones = np.ones((128, 128), dtype=np.float32)
    in_maps = []
    for c in range(NCORES):
        qsl = slice(c * G * D, (c + 1) * G * D)
        ksl = slice(c * D, (c + 1) * D)
        in_maps.append({
            "xt": xt,
            "wqt": np.ascontiguousarray(wq[qsl, :].T),
            "wkt": np.ascontiguousarray(wk[ksl, :].T),
            "wvt": np.ascontiguousarray(wv[ksl, :].T),
            "wot": np.ascontiguousarray(wo[:, qsl].T),
            "ones": ones,
        })
    return in_maps


def kernel(x, wq, wk, wv, wo, **run_kwargs):
    nc = _get_nc()
    in_maps = make_in_maps(np.asarray(x, dtype=np.float32),
                           np.asarray(wq, dtype=np.float32),
                           np.asarray(wk, dtype=np.float32),
                           np.asarray(wv, dtype=np.float32),
                           np.asarray(wo, dtype=np.float32))
    res = run_bass_kernel_spmd(nc, in_maps, core_ids=list(range(NCORES)),
                               **run_kwargs)
    acc = np.zeros((T, H), dtype=np.float32)
    for c in range(NCORES):
        acc += res.results[c]["out"]
    out = acc.reshape(B, S, H)
    if run_kwargs:
        return out, res
    return out



# revision 5
# speedup vs baseline: 1.2322x; 1.2322x over previous
"""Trainium2 Bass kernel for Llama GQA attention (no mask), 8-way tensor
parallel over KV heads.

Problem shapes (hardcoded):
  x  (2, 2048, 4096) f32
  wq (4096, 4096), wk (1024, 4096), wv (1024, 4096), wo (4096, 4096) f32
  NUM_HEADS=32, NUM_KV_HEADS=8, HEAD_DIM=128, GQA group g=4

Sharding: core c owns KV head c (4 Q heads). x replicated (pre-transposed
to xT on host), wq/wk/wv sharded on output dim (pre-transposed host-side),
wo sharded on input dim. Each core computes a partial (4096, 4096) output
(its heads' contribution through wo); host sums the 8 partials.

All matmuls run in fp32r (full-rate fp32, HIGH mode single pass).

v2 changes vs v1 (each PE matmul slot costs ~272ns = ldweights 228 + 44
handoff regardless of dtype, so wins come from cutting SLOT COUNT and
keeping the in-order PE queue from ever waiting):
  - softmax denominator no longer burns a 512-cycle ones-matmul per p
    chunk (512 slots = ~139us). DVE accumulates the exp() chunks in SBUF
    (2x mode) and a single ones-matmul per (b,tq-chunk,head) reduces the
    partition dim.
  - PV matmuls are emitted LAG g-ticks behind their scores matmuls so the
    ACT exp latency hides under later scores work instead of stalling the
    in-order PE queue (was ~86us of bubbles).
  - outproj's 32 PSUM groups interleave the residual PV drain.
  - phase-1 q0 PSUM double-buffered to shorten the j-boundary stall.
"""

import sys
from collections import deque
from contextlib import ExitStack

import numpy as np

sys.path.insert(0, "/opt/trn_rl_repo")

import concourse.bass as bass  # noqa: E402
import concourse.tile as tile  # noqa: E402
from concourse import bacc, mybir  # noqa: E402
from concourse.bass_utils import run_bass_kernel_spmd  # noqa: E402
from concourse.masks import make_identity  # noqa: E402

NCORES = 8
B, S, H = 2, 2048, 4096
T = B * S                      # 4096 flattened tokens
D = 128                        # head dim
G = 4                          # q heads per core (GQA group)
HK = 32                        # h k-tiles (4096 / 128)
TT = T // 128                  # 32 token tiles
NJ = T // 512                  # 8 token chunks of 512
SJ = S // 512                  # 4 tq chunks per batch
SI = S // 128                  # 16 tk tiles per batch
SCALE = float(1.0 / np.sqrt(D))
LAG = 3                        # pv emission lag in g-ticks

F32 = mybir.dt.float32
F32R = mybir.dt.float32r
COPY = mybir.ActivationFunctionType.Copy
EXP = mybir.ActivationFunctionType.Exp


def build_nc():
    nc = bacc.Bacc("TRN2", target_bir_lowering=False, debug=False,
                   enable_asserts=True, num_devices=NCORES)
    xt = nc.declare_dram_parameter("xt", [H, T], F32R, isOutput=False)
    wqt = nc.declare_dram_parameter("wqt", [H, G * D], F32R, isOutput=False)
    wkt = nc.declare_dram_parameter("wkt", [H, D], F32R, isOutput=False)
    wvt = nc.declare_dram_parameter("wvt", [H, D], F32R, isOutput=False)
    wot = nc.declare_dram_parameter("wot", [G * D, H], F32R, isOutput=False)
    ones = nc.declare_dram_parameter("ones", [128, 128], F32R, isOutput=False)
    out = nc.declare_dram_parameter("out", [T, H], F32, isOutput=True)

    xt_r = xt.ap().rearrange("(k p) t -> p k t", p=128)     # [128, 32, T]
    wqt_r = wqt.ap().rearrange("(k p) m -> p k m", p=128)   # [128, 32, 512]
    wkt_r = wkt.ap().rearrange("(k p) m -> p k m", p=128)   # [128, 32, 128]
    wvt_r = wvt.ap().rearrange("(k p) m -> p k m", p=128)   # [128, 32, 128]
    wot_r = wot.ap().rearrange("(k p) n -> p k n", p=128)   # [128, 4, T]
    out_r = out.ap()

    with tile.TileContext(nc) as tc:
        with ExitStack() as ctx:
            persist = ctx.enter_context(tc.tile_pool(name="persist", bufs=1))
            q_sb = persist.tile([128, G, T], F32R)       # qT per head, 8MB
            k_sb = persist.tile([128, T], F32R)          # kT, 2MB
            v_sb = persist.tile([128, TT, D], F32R)      # v natural, 2MB
            ones_sb = persist.tile([128, 128], F32R)
            nc.sync.dma_start(out=ones_sb, in_=ones.ap())

            # ---------------- phase 1: projections ----------------
            with ExitStack() as c1:
                wpool = c1.enter_context(tc.tile_pool(name="wpool", bufs=1))
                xpool = c1.enter_context(tc.tile_pool(name="xpool", bufs=4))
                vstg = c1.enter_context(tc.tile_pool(name="vstg", bufs=2))
                psq0 = c1.enter_context(tc.tile_pool(name="psq0", bufs=2, space="PSUM"))
                ps1 = c1.enter_context(tc.tile_pool(name="ps1", bufs=1, space="PSUM"))
                pstr = c1.enter_context(tc.tile_pool(name="pstr", bufs=1, space="PSUM"))

                wq_t = wpool.tile([128, HK, G * D], F32R)   # 8MB
                wk_t = wpool.tile([128, HK, D], F32R)       # 2MB
                wv_t = wpool.tile([128, HK, D], F32R)       # 2MB
                ident = wpool.tile([128, 128], F32)
                # chunk weight loads per k-tile on the gpsimd queue so the
                # first matmul's stationary arrives within ~1us
                for k in range(HK):
                    nc.gpsimd.dma_start(out=wq_t[:, k, :], in_=wqt_r[:, k, :])
                    nc.gpsimd.dma_start(out=wk_t[:, k, :], in_=wkt_r[:, k, :])
                    nc.gpsimd.dma_start(out=wv_t[:, k, :], in_=wvt_r[:, k, :])
                make_identity(nc, ident)

                def v_transpose(pj, pv_st):
                    # one-j-delayed so PE never waits on the DVE staging copy
                    vt_ps = pstr.tile([128, 4, 128], F32)
                    for tt in range(4):
                        nc.tensor.transpose(
                            vt_ps[:, tt, :], pv_st[:, tt * 128:(tt + 1) * 128],
                            ident)
                    nc.scalar.activation(
                        out=v_sb[:, 4 * pj:4 * pj + 4, :], in_=vt_ps, func=COPY)

                prev_v = None
                for j in range(NJ):
                    tsl = slice(j * 512, (j + 1) * 512)
                    q_ps = [psq0.tile([128, 512], F32, name="q_ps0")] + [
                        ps1.tile([128, 512], F32, name=f"q_ps{m}")
                        for m in range(1, G)]
                    k_ps = ps1.tile([128, 512], F32)
                    v_ps = ps1.tile([128, 512], F32)
                    for k in range(HK):
                        x_t = xpool.tile([128, 512], F32R)
                        nc.sync.dma_start(out=x_t, in_=xt_r[:, k, tsl])
                        st = k == 0
                        sp = k == HK - 1
                        for m in range(G):
                            nc.tensor.matmul(
                                q_ps[m], wq_t[:, k, m * D:(m + 1) * D], x_t,
                                start=st, stop=sp)
                        nc.tensor.matmul(k_ps, wk_t[:, k, :], x_t, start=st, stop=sp)
                        nc.tensor.matmul(v_ps, wv_t[:, k, :], x_t, start=st, stop=sp)
                        if k == 2 and prev_v is not None:
                            v_transpose(*prev_v)
                    # split psum evacuation across ACT and DVE so the banks
                    # free up fast for the next j iteration
                    nc.scalar.activation(out=q_sb[:, 0, tsl], in_=q_ps[0], func=COPY)
                    nc.vector.tensor_copy(q_sb[:, 1, tsl], q_ps[1])
                    nc.scalar.activation(out=q_sb[:, 2, tsl], in_=q_ps[2], func=COPY)
                    nc.vector.tensor_copy(q_sb[:, 3, tsl], q_ps[3])
                    nc.scalar.activation(out=k_sb[:, tsl], in_=k_ps, func=COPY)
                    # v: vT [dv, t] -> transpose 128-col blocks -> v [t, dv]
                    v_st = vstg.tile([128, 512], F32)
                    nc.vector.tensor_copy(v_st, v_ps)
                    prev_v = (j, v_st)
                v_transpose(*prev_v)

            # ------- phase 2: fused attention + output projection -------
            with ExitStack() as c2:
                wopool = c2.enter_context(tc.tile_pool(name="wopool", bufs=1))
                apool = c2.enter_context(tc.tile_pool(name="apool", bufs=2))
                ppool = c2.enter_context(tc.tile_pool(name="ppool", bufs=4))
                dpool = c2.enter_context(tc.tile_pool(name="dpool", bufs=2))
                rpool = c2.enter_context(tc.tile_pool(name="rpool", bufs=1))
                opool = c2.enter_context(tc.tile_pool(name="opool", bufs=3))
                psS = c2.enter_context(tc.tile_pool(name="psS", bufs=2, space="PSUM"))
                psPV = c2.enter_context(tc.tile_pool(name="psPV", bufs=2, space="PSUM"))
                psO = c2.enter_context(tc.tile_pool(name="psO", bufs=2, space="PSUM"))

                wo_sb = wopool.tile([128, G, T], F32R)      # 8MB resident
                for k in range(G):
                    nc.gpsimd.dma_start(out=wo_sb[:, k, :], in_=wot_r[:, k, :])

                # pending pv-pair closures, emitted LAG g-ticks late so the
                # in-order PE queue never waits on ACT's exp
                pending = deque()

                def drain(n):
                    for _ in range(min(n, len(pending))):
                        pending.popleft()()

                def outproj(bj):
                    # 32 psum groups; interleave the pv backlog so attention
                    # keeps flowing while outproj owns the PE
                    pb, pj, pa = bj
                    for tt2 in range(4):
                        t0 = pb * S + pj * 512 + tt2 * 128
                        for n in range(NJ):
                            o_ps = psO.tile([128, 512], F32)
                            for m in range(G):
                                nc.tensor.matmul(
                                    o_ps, pa[m][:, tt2 * 128:(tt2 + 1) * 128],
                                    wo_sb[:, m, n * 512:(n + 1) * 512],
                                    start=(m == 0), stop=(m == G - 1))
                            o_t = opool.tile([128, 512], F32)
                            if n % 2 == 0:
                                nc.scalar.activation(out=o_t, in_=o_ps, func=COPY)
                            else:
                                nc.vector.tensor_copy(o_t, o_ps)
                            nc.sync.dma_start(
                                out=out_r[t0:t0 + 128, n * 512:(n + 1) * 512],
                                in_=o_t)
                            drain(1)

                prev = None
                for b in range(B):
                    for j in range(SJ):
                        tqsl = slice(b * S + j * 512, b * S + (j + 1) * 512)
                        a_ch = [apool.tile([128, 512], F32R, name=f"a_ch{m}")
                                for m in range(G)]
                        for m in range(G):
                            pv_ps = psPV.tile([128, 512], F32)
                            den_acc = dpool.tile([128, 512], F32R)
                            for g in range(SI // 2):
                                s_ps = psS.tile([128, 1024], F32)
                                for h in range(2):
                                    ti = b * SI + 2 * g + h
                                    nc.tensor.matmul(
                                        s_ps[:, h * 512:(h + 1) * 512],
                                        k_sb[:, ti * 128:(ti + 1) * 128],
                                        q_sb[:, m, tqsl], start=True, stop=True)
                                p_t = ppool.tile([128, 1024], F32R)
                                nc.scalar.activation(out=p_t, in_=s_ps, func=EXP,
                                                     scale=SCALE)
                                # denominator: accumulate exp chunks on DVE
                                # (SBUF-only operands -> 2x mode)
                                if g == 0:
                                    nc.vector.tensor_copy(den_acc, p_t[:, 0:512])
                                else:
                                    nc.vector.tensor_add(den_acc, den_acc,
                                                         p_t[:, 0:512])
                                nc.vector.tensor_add(den_acc, den_acc,
                                                     p_t[:, 512:1024])

                                def pv_pair(g=g, p_t=p_t, pv_ps=pv_ps, b=b,
                                            m=m, den_acc=den_acc, a_m=a_ch[m]):
                                    for h2 in range(2):
                                        ti2 = b * SI + 2 * g + h2
                                        nc.tensor.matmul(
                                            pv_ps, v_sb[:, ti2, :],
                                            p_t[:, h2 * 512:(h2 + 1) * 512],
                                            start=(g == 0 and h2 == 0),
                                            stop=(g == SI // 2 - 1 and h2 == 1))
                                    if g == SI // 2 - 1:
                                        # finalize head: one ones-matmul sums
                                        # the partition dim, reciprocal + scale
                                        # on DVE
                                        den_ps = psO.tile([128, 512], F32,
                                                          name="o_ps")
                                        nc.tensor.matmul(den_ps, ones_sb,
                                                         den_acc,
                                                         start=True, stop=True)
                                        rec_t = rpool.tile([128, 512], F32)
                                        nc.vector.reciprocal_approx_fast(
                                            out=rec_t, in_=den_ps)
                                        nc.vector.tensor_mul(a_m, pv_ps, rec_t)

                                pending.append(pv_pair)
                                while len(pending) > LAG:
                                    pending.popleft()()
                        if prev is not None:
                            outproj(prev)
                        prev = (b, j, a_ch)
                drain(len(pending))
                outproj(prev)
    nc.compile()
    return nc


_NC_CACHE = None


def _get_nc():
    global _NC_CACHE
    if _NC_CACHE is None:
        _NC_CACHE = build_nc()
    return _NC_CACHE


def make_in_maps(x, wq, wk, wv, wo):
    xt = np.ascontiguousarray(x.reshape(T, H).T)
    ones = np.ones((128, 128), dtype=np.float32)
    in_maps = []
    for c in range(NCORES):
        qsl = slice(c * G * D, (c + 1) * G * D)
        ksl = slice(c * D, (c + 1) * D)
        in_maps.append({
            "xt": xt,
            "wqt": np.ascontiguousarray(wq[qsl, :].T),
            "wkt": np.ascontiguousarray(wk[ksl, :].T),
            "wvt": np.ascontiguousarray(wv[ksl, :].T),
            "wot": np.ascontiguousarray(wo[:, qsl].T),
            "ones": ones,
        })
    return in_maps


def kernel(x, wq, wk, wv, wo, **run_kwargs):
    nc = _get_nc()
    in_maps = make_in_maps(np.asarray(x, dtype=np.float32),
                           np.asarray(wq, dtype=np.float32),
                           np.asarray(wk, dtype=np.float32),
                           np.asarray(wv, dtype=np.float32),
                           np.asarray(wo, dtype=np.float32))
    res = run_bass_kernel_spmd(nc, in_maps, core_ids=list(range(NCORES)),
                               **run_kwargs)
    acc = np.zeros((T, H), dtype=np.float32)
    for c in range(NCORES):
        acc += res.results[c]["out"]
    out = acc.reshape(B, S, H)
    if run_kwargs:
        return out, res
    return out


# revision 8
# speedup vs baseline: 1.2624x; 1.0245x over previous
"""Trainium2 Bass kernel for Llama GQA attention (no mask), 8-way tensor
parallel over KV heads.

Problem shapes (hardcoded):
  x  (2, 2048, 4096) f32
  wq (4096, 4096), wk (1024, 4096), wv (1024, 4096), wo (4096, 4096) f32
  NUM_HEADS=32, NUM_KV_HEADS=8, HEAD_DIM=128, GQA group g=4

Sharding: core c owns KV head c (4 Q heads). x replicated (pre-transposed
to xT on host), wq/wk/wv sharded on output dim (pre-transposed host-side),
wo sharded on input dim. Each core computes a partial (4096, 4096) output
(its heads' contribution through wo); host sums the 8 partials.

All matmuls run in fp32r (full-rate fp32, HIGH mode single pass).

v2 changes vs v1 (each PE matmul slot costs ~272ns = ldweights 228 + 44
handoff regardless of dtype, so wins come from cutting SLOT COUNT and
keeping the in-order PE queue from ever waiting):
  - softmax denominator no longer burns a 512-cycle ones-matmul per p
    chunk (512 slots = ~139us). DVE accumulates the exp() chunks in SBUF
    (2x mode) and a single ones-matmul per (b,tq-chunk,head) reduces the
    partition dim.
  - PV matmuls are emitted LAG g-ticks behind their scores matmuls so the
    ACT exp latency hides under later scores work instead of stalling the
    in-order PE queue (was ~86us of bubbles).
  - outproj's 32 PSUM groups interleave the residual PV drain.
  - phase-1 q0 PSUM double-buffered to shorten the j-boundary stall.
"""

import sys
from collections import deque
from contextlib import ExitStack

import numpy as np

sys.path.insert(0, "/opt/trn_rl_repo")

import concourse.bass as bass  # noqa: E402
import concourse.tile as tile  # noqa: E402
from concourse import bacc, mybir  # noqa: E402
from concourse.bass_utils import run_bass_kernel_spmd  # noqa: E402
from concourse.masks import make_identity  # noqa: E402

NCORES = 8
B, S, H = 2, 2048, 4096
T = B * S                      # 4096 flattened tokens
D = 128                        # head dim
G = 4                          # q heads per core (GQA group)
HK = 32                        # h k-tiles (4096 / 128)
TT = T // 128                  # 32 token tiles
NJ = T // 512                  # 8 token chunks of 512
SJ = S // 512                  # 4 tq chunks per batch
SI = S // 128                  # 16 tk tiles per batch
SCALE = float(1.0 / np.sqrt(D))
LAG = 3                        # pv emission lag in g-ticks

F32 = mybir.dt.float32
F32R = mybir.dt.float32r
COPY = mybir.ActivationFunctionType.Copy
EXP = mybir.ActivationFunctionType.Exp


def build_nc():
    nc = bacc.Bacc("TRN2", target_bir_lowering=False, debug=False,
                   enable_asserts=True, num_devices=NCORES)
    xt = nc.declare_dram_parameter("xt", [H, T], F32R, isOutput=False)
    wqt = nc.declare_dram_parameter("wqt", [H, G * D], F32R, isOutput=False)
    wkt = nc.declare_dram_parameter("wkt", [H, D], F32R, isOutput=False)
    wvt = nc.declare_dram_parameter("wvt", [H, D], F32R, isOutput=False)
    wot = nc.declare_dram_parameter("wot", [G * D, H], F32R, isOutput=False)
    ones = nc.declare_dram_parameter("ones", [128, 128], F32R, isOutput=False)
    out = nc.declare_dram_parameter("out", [T, H], F32, isOutput=True)

    xt_r = xt.ap().rearrange("(k p) t -> p k t", p=128)     # [128, 32, T]
    wqt_r = wqt.ap().rearrange("(k p) m -> p k m", p=128)   # [128, 32, 512]
    wkt_r = wkt.ap().rearrange("(k p) m -> p k m", p=128)   # [128, 32, 128]
    wvt_r = wvt.ap().rearrange("(k p) m -> p k m", p=128)   # [128, 32, 128]
    wot_r = wot.ap().rearrange("(k p) n -> p k n", p=128)   # [128, 4, T]
    out_r = out.ap()

    with tile.TileContext(nc) as tc:
        with ExitStack() as ctx:
            persist = ctx.enter_context(tc.tile_pool(name="persist", bufs=1))
            q_sb = persist.tile([128, G, T], F32R)       # qT per head, 8MB
            k_sb = persist.tile([128, T], F32R)          # kT, 2MB
            v_sb = persist.tile([128, TT, D], F32R)      # v natural, 2MB
            ones_sb = persist.tile([128, 128], F32R)
            nc.sync.dma_start(out=ones_sb, in_=ones.ap())

            # ---------------- phase 1: projections ----------------
            with ExitStack() as c1:
                wpool = c1.enter_context(tc.tile_pool(name="wpool", bufs=1))
                xpool = c1.enter_context(tc.tile_pool(name="xpool", bufs=4))
                vstg = c1.enter_context(tc.tile_pool(name="vstg", bufs=2))
                psq0 = c1.enter_context(tc.tile_pool(name="psq0", bufs=2, space="PSUM"))
                ps1 = c1.enter_context(tc.tile_pool(name="ps1", bufs=1, space="PSUM"))
                pstr = c1.enter_context(tc.tile_pool(name="pstr", bufs=1, space="PSUM"))

                wq_t = wpool.tile([128, HK, G * D], F32R)   # 8MB
                wk_t = wpool.tile([128, HK, D], F32R)       # 2MB
                wv_t = wpool.tile([128, HK, D], F32R)       # 2MB
                ident = wpool.tile([128, 128], F32)
                # chunk weight loads per k-tile on the gpsimd queue so the
                # first matmul's stationary arrives within ~1us
                for k in range(HK):
                    nc.gpsimd.dma_start(out=wq_t[:, k, :], in_=wqt_r[:, k, :])
                    nc.gpsimd.dma_start(out=wk_t[:, k, :], in_=wkt_r[:, k, :])
                    nc.gpsimd.dma_start(out=wv_t[:, k, :], in_=wvt_r[:, k, :])
                make_identity(nc, ident)

                def v_transpose(pj, pv_st):
                    # one-j-delayed so PE never waits on the DVE staging copy
                    vt_ps = pstr.tile([128, 4, 128], F32)
                    for tt in range(4):
                        nc.tensor.transpose(
                            vt_ps[:, tt, :], pv_st[:, tt * 128:(tt + 1) * 128],
                            ident)
                    nc.scalar.activation(
                        out=v_sb[:, 4 * pj:4 * pj + 4, :], in_=vt_ps, func=COPY)

                prev_v = None
                for j in range(NJ):
                    tsl = slice(j * 512, (j + 1) * 512)
                    q_ps = [psq0.tile([128, 512], F32, name="q_ps0")] + [
                        ps1.tile([128, 512], F32, name=f"q_ps{m}")
                        for m in range(1, G)]
                    k_ps = ps1.tile([128, 512], F32)
                    v_ps = ps1.tile([128, 512], F32)
                    for k in range(HK):
                        x_t = xpool.tile([128, 512], F32R)
                        nc.sync.dma_start(out=x_t, in_=xt_r[:, k, tsl])
                        st = k == 0
                        sp = k == HK - 1
                        for m in range(G):
                            nc.tensor.matmul(
                                q_ps[m], wq_t[:, k, m * D:(m + 1) * D], x_t,
                                start=st, stop=sp)
                        nc.tensor.matmul(k_ps, wk_t[:, k, :], x_t, start=st, stop=sp)
                        nc.tensor.matmul(v_ps, wv_t[:, k, :], x_t, start=st, stop=sp)
                        if k == 2 and prev_v is not None:
                            v_transpose(*prev_v)
                    # split psum evacuation across ACT and DVE so the banks
                    # free up fast for the next j iteration
                    nc.scalar.activation(out=q_sb[:, 0, tsl], in_=q_ps[0], func=COPY)
                    nc.vector.tensor_copy(q_sb[:, 1, tsl], q_ps[1])
                    nc.scalar.activation(out=q_sb[:, 2, tsl], in_=q_ps[2], func=COPY)
                    nc.vector.tensor_copy(q_sb[:, 3, tsl], q_ps[3])
                    nc.scalar.activation(out=k_sb[:, tsl], in_=k_ps, func=COPY)
                    # v: vT [dv, t] -> transpose 128-col blocks -> v [t, dv]
                    v_st = vstg.tile([128, 512], F32)
                    nc.vector.tensor_copy(v_st, v_ps)
                    prev_v = (j, v_st)
                v_transpose(*prev_v)

            # ------- phase 2: fused attention + output projection -------
            with ExitStack() as c2:
                wopool = c2.enter_context(tc.tile_pool(name="wopool", bufs=1))
                apool = c2.enter_context(tc.tile_pool(name="apool", bufs=2))
                ppool = c2.enter_context(tc.tile_pool(name="ppool", bufs=3))
                dpool = c2.enter_context(tc.tile_pool(name="dpool", bufs=2))
                rpool = c2.enter_context(tc.tile_pool(name="rpool", bufs=1))
                opool = c2.enter_context(tc.tile_pool(name="opool", bufs=6))
                psS = c2.enter_context(tc.tile_pool(name="psS", bufs=2, space="PSUM"))
                psPV = c2.enter_context(tc.tile_pool(name="psPV", bufs=2, space="PSUM"))
                psO = c2.enter_context(tc.tile_pool(name="psO", bufs=2, space="PSUM"))

                wo_sb = wopool.tile([128, G, T], F32R)      # 8MB resident
                for k in range(G):
                    nc.gpsimd.dma_start(out=wo_sb[:, k, :], in_=wot_r[:, k, :])

                # pending pv-pair closures, emitted LAG g-ticks late so the
                # in-order PE queue never waits on ACT's exp
                pending = deque()

                def drain(n):
                    for _ in range(min(n, len(pending))):
                        pending.popleft()()

                def op_group(pa, pb, pj, tt2, n):
                    # one outproj psum group: 4 matmuls + evac + DMA. Fused
                    # into the attention tick loop (one group per g-tick) so
                    # outproj work fills the PE slack under ACT's exp latency
                    # and evac/DMA latencies spread out.
                    t0 = pb * S + pj * 512 + tt2 * 128
                    o_ps = psO.tile([128, 512], F32, name="o_ps")
                    for m in range(G):
                        nc.tensor.matmul(
                            o_ps, pa[m][:, tt2 * 128:(tt2 + 1) * 128],
                            wo_sb[:, m, n * 512:(n + 1) * 512],
                            start=(m == 0), stop=(m == G - 1))
                    o_t = opool.tile([128, 512], F32)
                    if n % 2 == 0:
                        nc.scalar.activation(out=o_t, in_=o_ps, func=COPY)
                    else:
                        nc.vector.tensor_copy(o_t, o_ps)
                    nc.sync.dma_start(
                        out=out_r[t0:t0 + 128, n * 512:(n + 1) * 512],
                        in_=o_t)

                prev = None
                for b in range(B):
                    for j in range(SJ):
                        # flush the attention pipeline: every a_ch write of
                        # the previous (b,j) must be emitted before outproj
                        # groups that read it (in-order PE queue)
                        drain(len(pending))
                        ops = deque()
                        if prev is not None:
                            pa, pb, pj = prev
                            for tt2 in range(4):
                                for n in range(NJ):
                                    ops.append((pa, pb, pj, tt2, n))
                        tqsl = slice(b * S + j * 512, b * S + (j + 1) * 512)
                        a_ch = [apool.tile([128, 512], F32R, name=f"a_ch{m}")
                                for m in range(G)]
                        for m in range(G):
                            pv_ps = psPV.tile([128, 512], F32)
                            den_acc = dpool.tile([128, 512], F32R)
                            for g in range(SI // 2):
                                s_ps = psS.tile([128, 1024], F32)
                                for h in range(2):
                                    ti = b * SI + 2 * g + h
                                    nc.tensor.matmul(
                                        s_ps[:, h * 512:(h + 1) * 512],
                                        k_sb[:, ti * 128:(ti + 1) * 128],
                                        q_sb[:, m, tqsl], start=True, stop=True)

                                def pv_pair(g=g, pv_ps=pv_ps, b=b, m=m,
                                            den_acc=den_acc, a_m=a_ch[m],
                                            p_t=None):
                                    for h2 in range(2):
                                        ti2 = b * SI + 2 * g + h2
                                        nc.tensor.matmul(
                                            pv_ps, v_sb[:, ti2, :],
                                            p_t[:, h2 * 512:(h2 + 1) * 512],
                                            start=(g == 0 and h2 == 0),
                                            stop=(g == SI // 2 - 1 and h2 == 1))
                                    if g == SI // 2 - 1:
                                        # finalize head: one ones-matmul sums
                                        # the partition dim, reciprocal + scale
                                        # on DVE
                                        den_ps = psO.tile([128, 512], F32,
                                                          name="o_ps")
                                        nc.tensor.matmul(den_ps, ones_sb,
                                                         den_acc,
                                                         start=True, stop=True)
                                        rec_t = rpool.tile([128, 512], F32)
                                        nc.vector.reciprocal_approx_fast(
                                            out=rec_t, in_=den_ps)
                                        nc.vector.tensor_mul(a_m, pv_ps, rec_t)

                                # pv for tick g-LAG runs before exp(g) is
                                # emitted: it frees the p_t slot exp(g) reuses
                                drain(1 if len(pending) >= LAG else 0)

                                p_t = ppool.tile([128, 1024], F32R)
                                nc.scalar.activation(out=p_t, in_=s_ps, func=EXP,
                                                     scale=SCALE)
                                # denominator: accumulate exp chunks on DVE
                                if g == 0:
                                    nc.vector.tensor_copy(den_acc, p_t[:, 0:512])
                                else:
                                    nc.vector.tensor_add(den_acc, den_acc,
                                                         p_t[:, 0:512])
                                nc.vector.tensor_add(den_acc, den_acc,
                                                     p_t[:, 512:1024])

                                def pv_bound(pv_pair=pv_pair, p_t=p_t):
                                    pv_pair(p_t=p_t)
                                pending.append(pv_bound)

                                if ops:
                                    op_group(*ops.popleft())
                        prev = (a_ch, b, j)
                # tail: drain pipeline, then the last (b,j)'s outproj block
                drain(len(pending))
                pa, pb, pj = prev
                for tt2 in range(4):
                    for n in range(NJ):
                        op_group(pa, pb, pj, tt2, n)
    nc.compile()
    return nc


_NC_CACHE = None


def _get_nc():
    global _NC_CACHE
    if _NC_CACHE is None:
        _NC_CACHE = build_nc()
    return _NC_CACHE


def make_in_maps(x, wq, wk, wv, wo):
    xt = np.ascontiguousarray(x.reshape(T, H).T)
    ones = np.ones((128, 128), dtype=np.float32)
    in_maps = []
    for c in range(NCORES):
        qsl = slice(c * G * D, (c + 1) * G * D)
        ksl = slice(c * D, (c + 1) * D)
        in_maps.append({
            "xt": xt,
            "wqt": np.ascontiguousarray(wq[qsl, :].T),
            "wkt": np.ascontiguousarray(wk[ksl, :].T),
            "wvt": np.ascontiguousarray(wv[ksl, :].T),
            "wot": np.ascontiguousarray(wo[:, qsl].T),
            "ones": ones,
        })
    return in_maps


def kernel(x, wq, wk, wv, wo, **run_kwargs):
    nc = _get_nc()
    in_maps = make_in_maps(np.asarray(x, dtype=np.float32),
                           np.asarray(wq, dtype=np.float32),
                           np.asarray(wk, dtype=np.float32),
                           np.asarray(wv, dtype=np.float32),
                           np.asarray(wo, dtype=np.float32))
    res = run_bass_kernel_spmd(nc, in_maps, core_ids=list(range(NCORES)),
                               **run_kwargs)
    acc = np.zeros((T, H), dtype=np.float32)
    for c in range(NCORES):
        acc += res.results[c]["out"]
    out = acc.reshape(B, S, H)
    if run_kwargs:
        return out, res
    return out


# revision 9
# speedup vs baseline: 1.4711x; 1.1654x over previous
"""Trainium2 Bass kernel for Llama GQA attention (no mask), 8-way tensor
parallel over KV heads.

Problem shapes (hardcoded):
  x  (2, 2048, 4096) f32
  wq (4096, 4096), wk (1024, 4096), wv (1024, 4096), wo (4096, 4096) f32
  NUM_HEADS=32, NUM_KV_HEADS=8, HEAD_DIM=128, GQA group g=4

Sharding: core c owns KV head c (4 Q heads). x replicated (pre-transposed
to xT on host), wq/wk/wv sharded on output dim (pre-transposed host-side),
wo sharded on input dim. Each core computes a partial (4096, 4096) f32
output (its heads' contribution through wo); host sums the 8 partials.

The whole matmul datapath runs in bf16 (PSUM accumulation stays fp32):
bf16 LDWEIGHTS takes ~97ns vs fp32r's ~187ns, which moves the per-matmul
cadence from ldweights-bound to stream-bound, and input DMA traffic
halves (x is 32MB/core instead of 64MB, which also cuts cross-core HBM
contention). Measured end-to-end rel err ~1e-3 vs the 2e-2 gate.

PE slot structure (one slot = ldweights + 512-col matmul, ~216-257ns):
  phase 1: 1536 projection slots + 32 bf16 transposes. Weights stream
    per-k-tile on the gpsimd queue; x streams 2 k-tiles per DMA on sync.
  phase 2: per (b, tq-chunk, head) unit: 16 scores + 16 pv + 1 ones-
    matmul (softmax denominator partition-sum; the per-chunk accumulation
    runs on DVE in 4x mode, off the PE). PV pairs are emitted LAG=3
    g-ticks behind their scores so ACT's exp latency never stalls the
    in-order PE queue, and one outproj psum group (4 matmuls of the
    previous (b, tq-chunk)) is fused into every g-tick to fill the
    remaining PE slack. Output rows batch into [128, 4096] tiles, one
    DMA per token tile.
"""

import sys
from collections import deque
from contextlib import ExitStack

import ml_dtypes
import numpy as np

sys.path.insert(0, "/opt/trn_rl_repo")

import concourse.bass as bass  # noqa: E402
import concourse.tile as tile  # noqa: E402
from concourse import bacc, mybir  # noqa: E402
from concourse.bass_utils import run_bass_kernel_spmd  # noqa: E402
from concourse.masks import make_identity  # noqa: E402

NCORES = 8
B, S, H = 2, 2048, 4096
T = B * S                      # 4096 flattened tokens
D = 128                        # head dim
G = 4                          # q heads per core (GQA group)
HK = 32                        # h k-tiles (4096 / 128)
TT = T // 128                  # 32 token tiles
NJ = T // 512                  # 8 token chunks of 512
SJ = S // 512                  # 4 tq chunks per batch
SI = S // 128                  # 16 tk tiles per batch
SCALE = float(1.0 / np.sqrt(D))
LAG = 3                        # pv emission lag in g-ticks

F32 = mybir.dt.float32
BF16 = mybir.dt.bfloat16
COPY = mybir.ActivationFunctionType.Copy
EXP = mybir.ActivationFunctionType.Exp
BF_NP = ml_dtypes.bfloat16


def build_nc():
    nc = bacc.Bacc("TRN2", target_bir_lowering=False, debug=False,
                   enable_asserts=True, num_devices=NCORES)
    xt = nc.declare_dram_parameter("xt", [H, T], BF16, isOutput=False)
    wqt = nc.declare_dram_parameter("wqt", [H, G * D], BF16, isOutput=False)
    wkt = nc.declare_dram_parameter("wkt", [H, D], BF16, isOutput=False)
    wvt = nc.declare_dram_parameter("wvt", [H, D], BF16, isOutput=False)
    wot = nc.declare_dram_parameter("wot", [G * D, H], BF16, isOutput=False)
    ones = nc.declare_dram_parameter("ones", [128, 128], BF16, isOutput=False)
    out = nc.declare_dram_parameter("out", [T, H], F32, isOutput=True)

    xt_r = xt.ap().rearrange("(k p) t -> p k t", p=128)     # [128, 32, T]
    wqt_r = wqt.ap().rearrange("(k p) m -> p k m", p=128)   # [128, 32, 512]
    wkt_r = wkt.ap().rearrange("(k p) m -> p k m", p=128)   # [128, 32, 128]
    wvt_r = wvt.ap().rearrange("(k p) m -> p k m", p=128)   # [128, 32, 128]
    wot_r = wot.ap().rearrange("(k p) n -> p k n", p=128)   # [128, 4, T]
    out_r = out.ap()

    with tile.TileContext(nc) as tc:
        with ExitStack() as ctx:
            persist = ctx.enter_context(tc.tile_pool(name="persist", bufs=1))
            q_sb = persist.tile([128, G, T], BF16)       # qT per head, 4MB
            k_sb = persist.tile([128, T], BF16)          # kT, 1MB
            v_sb = persist.tile([128, TT, D], BF16)      # v natural, 1MB
            ones_sb = persist.tile([128, 128], BF16)
            nc.sync.dma_start(out=ones_sb, in_=ones.ap())

            # ---------------- phase 1: projections ----------------
            with ExitStack() as c1:
                wpool = c1.enter_context(tc.tile_pool(name="wpool", bufs=1))
                xpool = c1.enter_context(tc.tile_pool(name="xpool", bufs=4))
                vstg = c1.enter_context(tc.tile_pool(name="vstg", bufs=2))
                psq0 = c1.enter_context(tc.tile_pool(name="psq0", bufs=2, space="PSUM"))
                ps1 = c1.enter_context(tc.tile_pool(name="ps1", bufs=1, space="PSUM"))
                pstr = c1.enter_context(tc.tile_pool(name="pstr", bufs=1, space="PSUM"))

                wq_t = wpool.tile([128, HK, G * D], BF16)   # 4MB
                wk_t = wpool.tile([128, HK, D], BF16)       # 1MB
                wv_t = wpool.tile([128, HK, D], BF16)       # 1MB
                ident = wpool.tile([128, 128], BF16)
                # chunk weight loads per k-tile on the gpsimd queue so the
                # first matmul's stationary arrives within ~1us
                for k in range(HK):
                    nc.gpsimd.dma_start(out=wq_t[:, k, :], in_=wqt_r[:, k, :])
                    nc.gpsimd.dma_start(out=wk_t[:, k, :], in_=wkt_r[:, k, :])
                    nc.gpsimd.dma_start(out=wv_t[:, k, :], in_=wvt_r[:, k, :])
                make_identity(nc, ident)

                def v_transpose(pj, pv_st):
                    # one-j-delayed so PE never waits on the DVE staging copy
                    vt_ps = pstr.tile([128, 4, 128], BF16)
                    for tt in range(4):
                        nc.tensor.transpose(
                            vt_ps[:, tt, :], pv_st[:, tt * 128:(tt + 1) * 128],
                            ident)
                    nc.scalar.activation(
                        out=v_sb[:, 4 * pj:4 * pj + 4, :], in_=vt_ps, func=COPY)

                prev_v = None
                for j in range(NJ):
                    tsl = slice(j * 512, (j + 1) * 512)
                    q_ps = [psq0.tile([128, 512], F32, name="q_ps0")] + [
                        ps1.tile([128, 512], F32, name=f"q_ps{m}")
                        for m in range(1, G)]
                    k_ps = ps1.tile([128, 512], F32)
                    v_ps = ps1.tile([128, 512], F32)
                    for k2 in range(HK // 2):
                        # two k-tiles per DMA: fewer, larger transfers
                        x_t = xpool.tile([128, 2, 512], BF16)
                        nc.sync.dma_start(out=x_t,
                                          in_=xt_r[:, 2 * k2:2 * k2 + 2, tsl])
                        for kk in range(2):
                            k = 2 * k2 + kk
                            st = k == 0
                            sp = k == HK - 1
                            for m in range(G):
                                nc.tensor.matmul(
                                    q_ps[m], wq_t[:, k, m * D:(m + 1) * D],
                                    x_t[:, kk, :], start=st, stop=sp)
                            nc.tensor.matmul(k_ps, wk_t[:, k, :], x_t[:, kk, :],
                                             start=st, stop=sp)
                            nc.tensor.matmul(v_ps, wv_t[:, k, :], x_t[:, kk, :],
                                             start=st, stop=sp)
                            if k == 2 and prev_v is not None:
                                v_transpose(*prev_v)
                    # split psum evacuation across ACT and DVE so the banks
                    # free up fast for the next j iteration
                    nc.scalar.activation(out=q_sb[:, 0, tsl], in_=q_ps[0], func=COPY)
                    nc.vector.tensor_copy(q_sb[:, 1, tsl], q_ps[1])
                    nc.scalar.activation(out=q_sb[:, 2, tsl], in_=q_ps[2], func=COPY)
                    nc.vector.tensor_copy(q_sb[:, 3, tsl], q_ps[3])
                    nc.scalar.activation(out=k_sb[:, tsl], in_=k_ps, func=COPY)
                    # v: vT [dv, t] -> transpose 128-col blocks -> v [t, dv]
                    v_st = vstg.tile([128, 512], BF16)
                    nc.vector.tensor_copy(v_st, v_ps)
                    prev_v = (j, v_st)
                v_transpose(*prev_v)

            # ------- phase 2: fused attention + output projection -------
            with ExitStack() as c2:
                wopool = c2.enter_context(tc.tile_pool(name="wopool", bufs=1))
                apool = c2.enter_context(tc.tile_pool(name="apool", bufs=2))
                ppool = c2.enter_context(tc.tile_pool(name="ppool", bufs=4))
                dpool = c2.enter_context(tc.tile_pool(name="dpool", bufs=2))
                rpool = c2.enter_context(tc.tile_pool(name="rpool", bufs=1))
                orow = c2.enter_context(tc.tile_pool(name="orow", bufs=2))
                psS = c2.enter_context(tc.tile_pool(name="psS", bufs=2, space="PSUM"))
                psPV = c2.enter_context(tc.tile_pool(name="psPV", bufs=2, space="PSUM"))
                psO = c2.enter_context(tc.tile_pool(name="psO", bufs=2, space="PSUM"))

                wo_sb = wopool.tile([128, G, T], BF16)      # 4MB resident
                for k in range(G):
                    nc.gpsimd.dma_start(out=wo_sb[:, k, :], in_=wot_r[:, k, :])

                # pending pv-pair closures, emitted LAG g-ticks late so the
                # in-order PE queue never waits on ACT's exp
                pending = deque()

                def drain(n):
                    for _ in range(min(n, len(pending))):
                        pending.popleft()()

                o_state = {}

                def op_group(pa, pb, pj, tt2, n):
                    # one outproj psum group: 4 matmuls + evac into a row
                    # batch tile; one DMA per completed [128, 4096] row.
                    # Fused into the attention tick loop (one group per
                    # g-tick) so outproj work fills the PE slack under ACT's
                    # exp latency and evac/DMA latencies spread out.
                    o_ps = psO.tile([128, 512], F32, name="o_ps")
                    for m in range(G):
                        nc.tensor.matmul(
                            o_ps, pa[m][:, tt2 * 128:(tt2 + 1) * 128],
                            wo_sb[:, m, n * 512:(n + 1) * 512],
                            start=(m == 0), stop=(m == G - 1))
                    if n == 0:
                        o_state['row'] = orow.tile([128, T], F32, name="o_row")
                    o_row = o_state['row']
                    if n % 2 == 0:
                        nc.scalar.activation(out=o_row[:, n * 512:(n + 1) * 512],
                                             in_=o_ps, func=COPY)
                    else:
                        nc.vector.tensor_copy(o_row[:, n * 512:(n + 1) * 512],
                                              o_ps)
                    if n == NJ - 1:
                        t0 = pb * S + pj * 512 + tt2 * 128
                        nc.sync.dma_start(out=out_r[t0:t0 + 128, :], in_=o_row)

                prev = None
                for b in range(B):
                    for j in range(SJ):
                        # flush the attention pipeline: every a_ch write of
                        # the previous (b,j) must be emitted before outproj
                        # groups that read it (in-order PE queue)
                        drain(len(pending))
                        ops = deque()
                        if prev is not None:
                            pa, pb, pj = prev
                            for tt2 in range(4):
                                for n in range(NJ):
                                    ops.append((pa, pb, pj, tt2, n))
                        tqsl = slice(b * S + j * 512, b * S + (j + 1) * 512)
                        a_ch = [apool.tile([128, 512], BF16, name=f"a_ch{m}")
                                for m in range(G)]
                        for m in range(G):
                            pv_ps = psPV.tile([128, 512], F32)
                            den_acc = dpool.tile([128, 512], BF16)
                            for g in range(SI // 2):
                                s_ps = psS.tile([128, 1024], F32)
                                for h in range(2):
                                    ti = b * SI + 2 * g + h
                                    nc.tensor.matmul(
                                        s_ps[:, h * 512:(h + 1) * 512],
                                        k_sb[:, ti * 128:(ti + 1) * 128],
                                        q_sb[:, m, tqsl], start=True, stop=True)

                                def pv_pair(g=g, pv_ps=pv_ps, b=b, m=m,
                                            den_acc=den_acc, a_m=a_ch[m],
                                            p_t=None):
                                    for h2 in range(2):
                                        ti2 = b * SI + 2 * g + h2
                                        nc.tensor.matmul(
                                            pv_ps, v_sb[:, ti2, :],
                                            p_t[:, h2 * 512:(h2 + 1) * 512],
                                            start=(g == 0 and h2 == 0),
                                            stop=(g == SI // 2 - 1 and h2 == 1))
                                    if g == SI // 2 - 1:
                                        # finalize head: one ones-matmul sums
                                        # the partition dim, reciprocal + scale
                                        # on DVE
                                        den_ps = psO.tile([128, 512], F32,
                                                          name="o_ps")
                                        nc.tensor.matmul(den_ps, ones_sb,
                                                         den_acc,
                                                         start=True, stop=True)
                                        rec_t = rpool.tile([128, 512], F32)
                                        nc.vector.reciprocal_approx_fast(
                                            out=rec_t, in_=den_ps)
                                        nc.vector.tensor_mul(a_m, pv_ps, rec_t)

                                # pv for tick g-LAG runs before exp(g) is
                                # emitted: it frees the p_t slot exp(g) reuses
                                drain(1 if len(pending) >= LAG else 0)

                                p_t = ppool.tile([128, 1024], BF16)
                                nc.scalar.activation(out=p_t, in_=s_ps, func=EXP,
                                                     scale=SCALE)
                                # denominator: accumulate exp chunks on DVE
                                # (bf16 + SBUF-only operands -> 4x mode)
                                if g == 0:
                                    nc.vector.tensor_copy(den_acc, p_t[:, 0:512])
                                else:
                                    nc.vector.tensor_add(den_acc, den_acc,
                                                         p_t[:, 0:512])
                                nc.vector.tensor_add(den_acc, den_acc,
                                                     p_t[:, 512:1024])

                                def pv_bound(pv_pair=pv_pair, p_t=p_t):
                                    pv_pair(p_t=p_t)
                                pending.append(pv_bound)

                                if ops:
                                    op_group(*ops.popleft())
                        prev = (a_ch, b, j)
                # tail: drain pipeline, then the last (b,j)'s outproj block
                drain(len(pending))
                pa, pb, pj = prev
                for tt2 in range(4):
                    for n in range(NJ):
                        op_group(pa, pb, pj, tt2, n)
    nc.compile()
    return nc


_NC_CACHE = None


def _get_nc():
    global _NC_CACHE
    if _NC_CACHE is None:
        _NC_CACHE = build_nc()
    return _NC_CACHE


def make_in_maps(x, wq, wk, wv, wo):
    xt = np.ascontiguousarray(x.reshape(T, H).T).astype(BF_NP)
    ones = np.ones((128, 128), dtype=BF_NP)
    in_maps = []
    for c in range(NCORES):
        qsl = slice(c * G * D, (c + 1) * G * D)
        ksl = slice(c * D, (c + 1) * D)
        in_maps.append({
            "xt": xt,
            "wqt": np.ascontiguousarray(wq[qsl, :].T).astype(BF_NP),
            "wkt": np.ascontiguousarray(wk[ksl, :].T).astype(BF_NP),
            "wvt": np.ascontiguousarray(wv[ksl, :].T).astype(BF_NP),
            "wot": np.ascontiguousarray(wo[:, qsl].T).astype(BF_NP),
            "ones": ones,
        })
    return in_maps


def kernel(x, wq, wk, wv, wo, **run_kwargs):
    nc = _get_nc()
    in_maps = make_in_maps(np.asarray(x, dtype=np.float32),
                           np.asarray(wq, dtype=np.float32),
                           np.asarray(wk, dtype=np.float32),
                           np.asarray(wv, dtype=np.float32),
                           np.asarray(wo, dtype=np.float32))
    res = run_bass_kernel_spmd(nc, in_maps, core_ids=list(range(NCORES)),
                               **run_kwargs)
    acc = np.zeros((T, H), dtype=np.float32)
    for c in range(NCORES):
        acc += res.results[c]["out"]
    out = acc.reshape(B, S, H)
    if run_kwargs:
        return out, res
    return out


# revision 12
# speedup vs baseline: 1.4770x; 1.0040x over previous
"""Trainium2 Bass kernel for Llama GQA attention (no mask), 8-way tensor
parallel over KV heads.

Problem shapes (hardcoded):
  x  (2, 2048, 4096) f32
  wq (4096, 4096), wk (1024, 4096), wv (1024, 4096), wo (4096, 4096) f32
  NUM_HEADS=32, NUM_KV_HEADS=8, HEAD_DIM=128, GQA group g=4

Sharding: core c owns KV head c (4 Q heads). x replicated (pre-transposed
to xT on host), wq/wk/wv sharded on output dim (pre-transposed host-side),
wo sharded on input dim. Each core computes a partial (4096, 4096) f32
output (its heads' contribution through wo); host sums the 8 partials.

The whole matmul datapath runs in bf16 (PSUM accumulation stays fp32):
bf16 LDWEIGHTS takes ~97ns vs fp32r's ~187ns, which moves the per-matmul
cadence from ldweights-bound to stream-bound, and input DMA traffic
halves (x is 32MB/core instead of 64MB, which also cuts cross-core HBM
contention). Measured end-to-end rel err ~1e-3 vs the 2e-2 gate.

PE slot structure (one slot = ldweights + 512-col matmul, ~216-257ns):
  phase 1: 1536 projection slots + 32 bf16 transposes. Weights stream
    per-k-tile on the gpsimd queue; x streams 2 k-tiles per DMA on sync.
  phase 2: per (b, tq-chunk, head) unit: 16 scores + 16 pv + 1 ones-
    matmul (softmax denominator partition-sum; the per-chunk accumulation
    runs on DVE in 4x mode, off the PE). PV pairs are emitted LAG=3
    g-ticks behind their scores so ACT's exp latency never stalls the
    in-order PE queue, and one outproj psum group (4 matmuls of the
    previous (b, tq-chunk)) is fused into every g-tick to fill the
    remaining PE slack. Output rows batch into [128, 4096] tiles, one
    DMA per token tile.
"""

import sys
from collections import deque
from contextlib import ExitStack

import ml_dtypes
import numpy as np

sys.path.insert(0, "/opt/trn_rl_repo")

import concourse.bass as bass  # noqa: E402
import concourse.tile as tile  # noqa: E402
from concourse import bacc, mybir  # noqa: E402
from concourse.bass_utils import run_bass_kernel_spmd  # noqa: E402
from concourse.masks import make_identity  # noqa: E402

NCORES = 8
B, S, H = 2, 2048, 4096
T = B * S                      # 4096 flattened tokens
D = 128                        # head dim
G = 4                          # q heads per core (GQA group)
HK = 32                        # h k-tiles (4096 / 128)
TT = T // 128                  # 32 token tiles
NJ = T // 512                  # 8 token chunks of 512
SJ = S // 512                  # 4 tq chunks per batch
SI = S // 128                  # 16 tk tiles per batch
SCALE = float(1.0 / np.sqrt(D))
LAG = 3                        # pv emission lag in g-ticks

F32 = mybir.dt.float32
BF16 = mybir.dt.bfloat16
COPY = mybir.ActivationFunctionType.Copy
EXP = mybir.ActivationFunctionType.Exp
BF_NP = ml_dtypes.bfloat16


def build_nc():
    nc = bacc.Bacc("TRN2", target_bir_lowering=False, debug=False,
                   enable_asserts=True, num_devices=NCORES)
    xt = nc.declare_dram_parameter("xt", [H, T], BF16, isOutput=False)
    wqt = nc.declare_dram_parameter("wqt", [H, G * D], BF16, isOutput=False)
    wkt = nc.declare_dram_parameter("wkt", [H, D], BF16, isOutput=False)
    wvt = nc.declare_dram_parameter("wvt", [H, D], BF16, isOutput=False)
    wot = nc.declare_dram_parameter("wot", [G * D, H], BF16, isOutput=False)
    ones = nc.declare_dram_parameter("ones", [128, 128], BF16, isOutput=False)
    # partial outputs ship as bf16 (summed in f32 on host): halves the
    # 64MB/core output stream
    out = nc.declare_dram_parameter("out", [T, H], BF16, isOutput=True)

    xt_r = xt.ap().rearrange("(k p) t -> p k t", p=128)     # [128, 32, T]
    wqt_r = wqt.ap().rearrange("(k p) m -> p k m", p=128)   # [128, 32, 512]
    wkt_r = wkt.ap().rearrange("(k p) m -> p k m", p=128)   # [128, 32, 128]
    wvt_r = wvt.ap().rearrange("(k p) m -> p k m", p=128)   # [128, 32, 128]
    wot_r = wot.ap().rearrange("(k p) n -> p k n", p=128)   # [128, 4, T]
    out_r = out.ap()

    with tile.TileContext(nc) as tc:
        with ExitStack() as ctx:
            persist = ctx.enter_context(tc.tile_pool(name="persist", bufs=1))
            q_sb = persist.tile([128, G, T], BF16)       # qT per head, 4MB
            k_sb = persist.tile([128, T], BF16)          # kT, 1MB
            v_sb = persist.tile([128, TT, D], BF16)      # v natural, 1MB
            ones_sb = persist.tile([128, 128], BF16)
            nc.sync.dma_start(out=ones_sb, in_=ones.ap())

            # ---------------- phase 1: projections ----------------
            with ExitStack() as c1:
                wpool = c1.enter_context(tc.tile_pool(name="wpool", bufs=1))
                xpool = c1.enter_context(tc.tile_pool(name="xpool", bufs=4))
                vstg = c1.enter_context(tc.tile_pool(name="vstg", bufs=2))
                psq0 = c1.enter_context(tc.tile_pool(name="psq0", bufs=2, space="PSUM"))
                ps1 = c1.enter_context(tc.tile_pool(name="ps1", bufs=1, space="PSUM"))
                pstr = c1.enter_context(tc.tile_pool(name="pstr", bufs=1, space="PSUM"))

                wq_t = wpool.tile([128, HK, G * D], BF16)   # 4MB
                wk_t = wpool.tile([128, HK, D], BF16)       # 1MB
                wv_t = wpool.tile([128, HK, D], BF16)       # 1MB
                ident = wpool.tile([128, 128], BF16)
                # chunk weight loads per k-tile on the gpsimd queue so the
                # first matmul's stationary arrives within ~1us
                for k in range(HK):
                    nc.gpsimd.dma_start(out=wq_t[:, k, :], in_=wqt_r[:, k, :])
                    nc.gpsimd.dma_start(out=wk_t[:, k, :], in_=wkt_r[:, k, :])
                    nc.gpsimd.dma_start(out=wv_t[:, k, :], in_=wvt_r[:, k, :])
                make_identity(nc, ident)

                def v_transpose(pj, pv_st):
                    # one-j-delayed so PE never waits on the DVE staging copy
                    vt_ps = pstr.tile([128, 4, 128], BF16)
                    for tt in range(4):
                        nc.tensor.transpose(
                            vt_ps[:, tt, :], pv_st[:, tt * 128:(tt + 1) * 128],
                            ident)
                    nc.scalar.activation(
                        out=v_sb[:, 4 * pj:4 * pj + 4, :], in_=vt_ps, func=COPY)

                prev_v = None
                for j in range(NJ):
                    tsl = slice(j * 512, (j + 1) * 512)
                    q_ps = [psq0.tile([128, 512], F32, name="q_ps0")] + [
                        ps1.tile([128, 512], F32, name=f"q_ps{m}")
                        for m in range(1, G)]
                    k_ps = ps1.tile([128, 512], F32)
                    v_ps = ps1.tile([128, 512], F32)
                    for k2 in range(HK // 2):
                        # two k-tiles per DMA: fewer, larger transfers
                        x_t = xpool.tile([128, 2, 512], BF16)
                        nc.sync.dma_start(out=x_t,
                                          in_=xt_r[:, 2 * k2:2 * k2 + 2, tsl])
                        for kk in range(2):
                            k = 2 * k2 + kk
                            st = k == 0
                            sp = k == HK - 1
                            for m in range(G):
                                nc.tensor.matmul(
                                    q_ps[m], wq_t[:, k, m * D:(m + 1) * D],
                                    x_t[:, kk, :], start=st, stop=sp)
                            nc.tensor.matmul(k_ps, wk_t[:, k, :], x_t[:, kk, :],
                                             start=st, stop=sp)
                            nc.tensor.matmul(v_ps, wv_t[:, k, :], x_t[:, kk, :],
                                             start=st, stop=sp)
                            if k == 2 and prev_v is not None:
                                v_transpose(*prev_v)
                    # split psum evacuation across ACT and DVE so the banks
                    # free up fast for the next j iteration
                    nc.scalar.activation(out=q_sb[:, 0, tsl], in_=q_ps[0], func=COPY)
                    nc.vector.tensor_copy(q_sb[:, 1, tsl], q_ps[1])
                    nc.scalar.activation(out=q_sb[:, 2, tsl], in_=q_ps[2], func=COPY)
                    nc.vector.tensor_copy(q_sb[:, 3, tsl], q_ps[3])
                    nc.scalar.activation(out=k_sb[:, tsl], in_=k_ps, func=COPY)
                    # v: vT [dv, t] -> transpose 128-col blocks -> v [t, dv]
                    v_st = vstg.tile([128, 512], BF16)
                    nc.vector.tensor_copy(v_st, v_ps)
                    prev_v = (j, v_st)
                v_transpose(*prev_v)

            # ------- phase 2: fused attention + output projection -------
            with ExitStack() as c2:
                wopool = c2.enter_context(tc.tile_pool(name="wopool", bufs=1))
                apool = c2.enter_context(tc.tile_pool(name="apool", bufs=2))
                ppool = c2.enter_context(tc.tile_pool(name="ppool", bufs=4))
                dpool = c2.enter_context(tc.tile_pool(name="dpool", bufs=2))
                rpool = c2.enter_context(tc.tile_pool(name="rpool", bufs=1))
                orow = c2.enter_context(tc.tile_pool(name="orow", bufs=2))
                psS = c2.enter_context(tc.tile_pool(name="psS", bufs=2, space="PSUM"))
                psPV = c2.enter_context(tc.tile_pool(name="psPV", bufs=2, space="PSUM"))
                psO = c2.enter_context(tc.tile_pool(name="psO", bufs=2, space="PSUM"))

                wo_sb = wopool.tile([128, G, T], BF16)      # 4MB resident
                for k in range(G):
                    nc.gpsimd.dma_start(out=wo_sb[:, k, :], in_=wot_r[:, k, :])

                # pending pv-pair closures, emitted LAG g-ticks late so the
                # in-order PE queue never waits on ACT's exp
                pending = deque()

                def drain(n):
                    for _ in range(min(n, len(pending))):
                        pending.popleft()()

                o_state = {}

                def op_group(pa, pb, pj, tt2, n):
                    # one outproj psum group: 4 matmuls + evac into a row
                    # batch tile; one DMA per completed [128, 4096] row.
                    # Fused into the attention tick loop (one group per
                    # g-tick) so outproj work fills the PE slack under ACT's
                    # exp latency and evac/DMA latencies spread out.
                    o_ps = psO.tile([128, 512], F32, name="o_ps")
                    for m in range(G):
                        nc.tensor.matmul(
                            o_ps, pa[m][:, tt2 * 128:(tt2 + 1) * 128],
                            wo_sb[:, m, n * 512:(n + 1) * 512],
                            start=(m == 0), stop=(m == G - 1))
                    if n == 0:
                        o_state['row'] = orow.tile([128, T], BF16, name="o_row")
                    o_row = o_state['row']
                    if n % 2 == 0:
                        nc.scalar.activation(out=o_row[:, n * 512:(n + 1) * 512],
                                             in_=o_ps, func=COPY)
                    else:
                        nc.vector.tensor_copy(o_row[:, n * 512:(n + 1) * 512],
                                              o_ps)
                    if n == NJ - 1:
                        t0 = pb * S + pj * 512 + tt2 * 128
                        nc.sync.dma_start(out=out_r[t0:t0 + 128, :], in_=o_row)

                prev = None
                for b in range(B):
                    for j in range(SJ):
                        # flush the attention pipeline: every a_ch write of
                        # the previous (b,j) must be emitted before outproj
                        # groups that read it (in-order PE queue)
                        drain(len(pending))
                        ops = deque()
                        if prev is not None:
                            pa, pb, pj = prev
                            for tt2 in range(4):
                                for n in range(NJ):
                                    ops.append((pa, pb, pj, tt2, n))
                        tqsl = slice(b * S + j * 512, b * S + (j + 1) * 512)
                        a_ch = [apool.tile([128, 512], BF16, name=f"a_ch{m}")
                                for m in range(G)]
                        for m in range(G):
                            pv_ps = psPV.tile([128, 512], F32)
                            den_acc = dpool.tile([128, 512], BF16)
                            for g in range(SI // 2):
                                s_ps = psS.tile([128, 1024], F32)
                                for h in range(2):
                                    ti = b * SI + 2 * g + h
                                    nc.tensor.matmul(
                                        s_ps[:, h * 512:(h + 1) * 512],
                                        k_sb[:, ti * 128:(ti + 1) * 128],
                                        q_sb[:, m, tqsl], start=True, stop=True)

                                def pv_pair(g=g, pv_ps=pv_ps, b=b, m=m,
                                            den_acc=den_acc, a_m=a_ch[m],
                                            p_t=None):
                                    for h2 in range(2):
                                        ti2 = b * SI + 2 * g + h2
                                        nc.tensor.matmul(
                                            pv_ps, v_sb[:, ti2, :],
                                            p_t[:, h2 * 512:(h2 + 1) * 512],
                                            start=(g == 0 and h2 == 0),
                                            stop=(g == SI // 2 - 1 and h2 == 1))
                                    if g == SI // 2 - 1:
                                        # finalize head: one ones-matmul sums
                                        # the partition dim, reciprocal + scale
                                        # on DVE
                                        den_ps = psO.tile([128, 512], F32,
                                                          name="o_ps")
                                        nc.tensor.matmul(den_ps, ones_sb,
                                                         den_acc,
                                                         start=True, stop=True)
                                        rec_t = rpool.tile([128, 512], F32)
                                        nc.vector.reciprocal_approx_fast(
                                            out=rec_t, in_=den_ps)
                                        nc.vector.tensor_mul(a_m, pv_ps, rec_t)

                                # pv for tick g-LAG runs before exp(g) is
                                # emitted: it frees the p_t slot exp(g) reuses
                                drain(1 if len(pending) >= LAG else 0)

                                p_t = ppool.tile([128, 1024], BF16)
                                nc.scalar.activation(out=p_t, in_=s_ps, func=EXP,
                                                     scale=SCALE)
                                # denominator: accumulate exp chunks on DVE
                                # (bf16 + SBUF-only operands -> 4x mode)
                                if g == 0:
                                    nc.vector.tensor_copy(den_acc, p_t[:, 0:512])
                                else:
                                    nc.vector.tensor_add(den_acc, den_acc,
                                                         p_t[:, 0:512])
                                nc.vector.tensor_add(den_acc, den_acc,
                                                     p_t[:, 512:1024])

                                def pv_bound(pv_pair=pv_pair, p_t=p_t):
                                    pv_pair(p_t=p_t)
                                pending.append(pv_bound)

                                if ops:
                                    op_group(*ops.popleft())
                        prev = (a_ch, b, j)
                # tail: drain pipeline, then the last (b,j)'s outproj block
                drain(len(pending))
                pa, pb, pj = prev
                for tt2 in range(4):
                    for n in range(NJ):
                        op_group(pa, pb, pj, tt2, n)
    nc.compile()
    return nc


_NC_CACHE = None


def _get_nc():
    global _NC_CACHE
    if _NC_CACHE is None:
        _NC_CACHE = build_nc()
    return _NC_CACHE


def make_in_maps(x, wq, wk, wv, wo):
    xt = np.ascontiguousarray(x.reshape(T, H).T).astype(BF_NP)
    ones = np.ones((128, 128), dtype=BF_NP)
    in_maps = []
    for c in range(NCORES):
        qsl = slice(c * G * D, (c + 1) * G * D)
        ksl = slice(c * D, (c + 1) * D)
        in_maps.append({
            "xt": xt,
            "wqt": np.ascontiguousarray(wq[qsl, :].T).astype(BF_NP),
            "wkt": np.ascontiguousarray(wk[ksl, :].T).astype(BF_NP),
            "wvt": np.ascontiguousarray(wv[ksl, :].T).astype(BF_NP),
            "wot": np.ascontiguousarray(wo[:, qsl].T).astype(BF_NP),
            "ones": ones,
        })
    return in_maps


def kernel(x, wq, wk, wv, wo, **run_kwargs):
    nc = _get_nc()
    in_maps = make_in_maps(np.asarray(x, dtype=np.float32),
                           np.asarray(wq, dtype=np.float32),
                           np.asarray(wk, dtype=np.float32),
                           np.asarray(wv, dtype=np.float32),
                           np.asarray(wo, dtype=np.float32))
    res = run_bass_kernel_spmd(nc, in_maps, core_ids=list(range(NCORES)),
                               **run_kwargs)
    acc = np.zeros((T, H), dtype=np.float32)
    for c in range(NCORES):
        acc += res.results[c]["out"].astype(np.float32)
    out = acc.reshape(B, S, H)
    if run_kwargs:
        return out, res
    return out
